# revision 1
# baseline (speedup 1.0000x reference)
"""GAT 2-layer encoder on 8 Trainium2 NeuronCores.

Reference computation: layer 1 = GAT conv over edge_index[:, :500] (weights W1),
layer 2 = GAT conv over edge_index[:, 500:] (weights W2).

Strategy:
  - Layer-1 output x1 differs from b1 only on the <=500 distinct dsts of the
    first 500 edges ("specials").  By linearity, layer 2's weighted aggregation
    commutes with the W2 transform, so layer 2 gathers x1-space rows and the
    gather table collapses to <=501 distinct 512B rows [x1 | asrc2 | adst2 | pad]
    (row 0 = default b1 row, rows 1..K = specials).  Indices then fit in int16
    for dma_gather.
  - Sharding: dst-range partition of the 1.6M layer-2 edges across 8 cores (no
    collectives; layer 1 + table build replicated on every core, it is tiny).
  - Per core: dsts sorted by in-degree, grouped into blocks of 128 (one dst per
    SBUF partition, its edges along the free dim, padded to the block max degree
    L).  One dma_gather per superblock fetches one 512B row per edge slot.
    Segment softmax = per-partition free-dim ops (DVE/ACT), weighted sum = DVE
    mul + strided reduce, final out = PE matmul [msgT;1] @ [W2;b2].
"""

import sys

sys.path.insert(0, "/opt/trn_rl_repo")

from contextlib import ExitStack

import numpy as np

import concourse.bacc as bacc
import concourse.bass as bass
import concourse.mybir as mybir
import concourse.tile as tile
from concourse.bass_utils import run_bass_kernel_spmd
from concourse.masks import make_identity

F32 = mybir.dt.float32
I16 = mybir.dt.int16
I32 = mybir.dt.int32
AF = mybir.ActivationFunctionType
OP = mybir.AluOpType

N = 100000
D = 64
NCORES = 8
NPC = N // NCORES          # dst nodes per core
P = 128
NSPLIT = 500               # first 500 edges -> layer 1
SMAX = 80                  # max edge-slots per superblock (SBUF budget)
NEG_SLOPE = 0.2
EPS = 1e-16
BIG = 200.0                # score shift so padded slots underflow exp to 0.0
GCHUNK = 32                # slots per packed gather call
PW = 4                     # slots packed per gather descriptor (PW*512B rows)


def _wrap16(flat):
    """int16 stream [n] (n%16==0) -> dma_gather idx tile [128, n//16]."""
    w = flat.reshape(-1, 16).T
    return np.ascontiguousarray(np.tile(w, (8, 1)).astype(np.int16))


def _grid(deg_sorted_max, npos):
    """Block structure from the (cross-core max) descending degree profile.

    Returns (L_b list, superblocks, groups):
      superblocks: dicts {b0, b1, S (slots), slot0}
      groups: dicts {sb, b0, B, L, slot_off (slots from sb start)}
    """
    nblocks = npos // P
    L = [max(int(deg_sorted_max[b * P]), 1) for b in range(nblocks)]
    sbs = []
    b = 0
    while b < nblocks:
        s = 0
        b0 = b
        while b < nblocks and (b - b0) < 16 and s + L[b] <= max(SMAX, L[b0]):
            s += L[b]
            b += 1
        sbs.append({"b0": b0, "b1": b, "S": s})
    slot0 = 0
    for sb in sbs:
        sb["slot0"] = slot0
        slot0 += sb["S"]
    groups = []
    for si, sb in enumerate(sbs):
        off = 0
        b = sb["b0"]
        while b < sb["b1"]:
            b0 = b
            while b < sb["b1"] and L[b] == L[b0]:
                b += 1
            groups.append({"sb": si, "b0": b0, "B": b - b0, "L": L[b0], "slot_off": off})
            off += (b - b0) * L[b0]
    return L, sbs, groups


VTAB = 1024               # gather table rows (specials + default replicas)


def _edge_streams(src, dst_local, rowmap_vals, npos, npc, Lb, sbs, repl_lo):
    """Per-partition edge grid for one core.

    Returns (eidx [128, 8*S_total] i16, mask [128, S_total] f32,
             degpos [128, nblocks] f32, order [npc])."""
    nblocks = npos // P
    deg = np.bincount(dst_local, minlength=npc)
    order = np.argsort(-deg, kind="stable")
    deg_sorted = deg[order]
    rank = np.empty(npc, np.int64)
    rank[order] = np.arange(npc)
    pos = rank[dst_local]
    pe = np.argsort(pos, kind="stable")
    pos_s = pos[pe]
    val_s = rowmap_vals[pe]
    start_of_pos = np.searchsorted(pos_s, np.arange(npos))
    k = np.arange(len(pos_s)) - start_of_pos[pos_s]
    blk = pos_s // P
    prt = pos_s % P
    slot_base = np.concatenate([[0], np.cumsum(Lb)])[:-1]
    s_global = slot_base[blk] + k
    S_total = int(sum(Lb))
    flat_j = s_global * P + prt
    rng = np.random.default_rng(12345)
    idxflat = rng.integers(repl_lo, VTAB, S_total * P).astype(np.int16)
    vs = val_s.astype(np.int16)
    zz = vs == 0
    vs[zz] = rng.integers(repl_lo, VTAB, int(zz.sum())).astype(np.int16)
    idxflat[flat_j] = vs
    maskflat = np.zeros(S_total * P, np.float32)
    maskflat[flat_j] = 1.0
    mask = np.ascontiguousarray(maskflat.reshape(S_total, P).T)
    eidx = np.concatenate(
        [_wrap16(idxflat[sb["slot0"] * P:(sb["slot0"] + sb["S"]) * P]) for sb in sbs],
        axis=1,
    )
    degpad = np.zeros(npos, np.float32)
    degpad[:npc] = deg_sorted
    degpos = np.ascontiguousarray((degpad > 0).astype(np.float32).reshape(nblocks, P).T)
    return eidx, mask, degpos, order, idxflat


def prep(inputs):
    """Host-side index prep (pure index computation, no feature values)."""
    ei = np.asarray(inputs["edge_index"])
    src = ei[0].astype(np.int64)
    dst = ei[1].astype(np.int64)
    s1, d1 = src[:NSPLIT], dst[:NSPLIT]
    s2, d2 = src[NSPLIT:], dst[NSPLIT:]

    # ---- layer 1 structure ----
    specials, deg1 = np.unique(d1, return_counts=True)
    K = len(specials)
    order1 = np.argsort(-deg1, kind="stable")
    spec_by_pos = specials[order1]          # grid position q -> node, table row q+1
    rowmap = np.zeros(N, np.int16)
    rowmap[spec_by_pos] = np.arange(1, K + 1)
    K1pos = K + 1                            # one guaranteed pad slot (default row)
    nblk1 = (K1pos + P - 1) // P
    npos1 = nblk1 * P

    U = np.unique(np.concatenate([s1, d1]))
    nU = len(U)
    nUt = (nU + P - 1) // P
    uidx = np.zeros((P, nUt), np.int32)
    upad = np.zeros(nUt * P, np.int64)
    upad[:nU] = U
    uidx[:, :] = upad.reshape(nUt, P).T
    uindex = np.zeros(N, np.int64)
    uindex[U] = np.arange(nU)

    # layer-1 edge grid (dst -> grid position via rank over specials)
    rank1 = np.empty(K, np.int64)
    rank1[order1] = np.arange(K)
    d1pos = rank1[np.searchsorted(specials, d1)]
    deg1_sorted = np.zeros(npos1, np.int64)
    deg1_sorted[:K] = deg1[order1]
    L1, sbs1, groups1 = _grid(deg1_sorted, npos1)
    S1 = int(sum(L1))
    # edge stream for layer 1 (single "core")
    pe = np.argsort(d1pos, kind="stable")
    pos_s = d1pos[pe]
    val_s = uindex[s1[pe]].astype(np.int16)
    start_of_pos = np.searchsorted(pos_s, np.arange(npos1))
    k = np.arange(len(pos_s)) - start_of_pos[pos_s]
    slot_base = np.concatenate([[0], np.cumsum(L1)])[:-1]
    flat_j = (slot_base[pos_s // P] + k) * P + (pos_s % P)
    idxflat = np.zeros(S1 * P, np.int16)
    idxflat[flat_j] = val_s
    maskflat = np.zeros(S1 * P, np.float32)
    maskflat[flat_j] = 1.0
    l1_mask = np.ascontiguousarray(maskflat.reshape(S1, P).T)
    l1_eidx = np.concatenate(
        [_wrap16(idxflat[sb["slot0"] * P:(sb["slot0"] + sb["S"]) * P]) for sb in sbs1],
        axis=1,
    )
    dv1 = np.zeros(npos1, np.int16)
    dv1[:K] = uindex[spec_by_pos]
    l1_didx = np.concatenate(
        [_wrap16(dv1[P * sb["b0"]:P * sb["b1"]]) for sb in sbs1], axis=1
    )
    dp1 = np.zeros(npos1, np.float32)
    dp1[:K] = (deg1[order1] > 0)
    l1_degpos = np.ascontiguousarray(dp1.reshape(nblk1, P).T)

    # ---- layer 2 structure ----
    npos = ((NPC + P - 1) // P) * P
    core_dat = []
    deg_sorted_all = np.zeros(npos, np.int64)
    for c in range(NCORES):
        sel = (d2 >= c * NPC) & (d2 < (c + 1) * NPC)
        dl = d2[sel] - c * NPC
        sl = s2[sel]
        deg = np.bincount(dl, minlength=NPC)
        ds = np.sort(deg)[::-1]
        m = min(NPC, npos)
        deg_sorted_all[:m] = np.maximum(deg_sorted_all[:m], ds[:m])
        core_dat.append((sl, dl))
    L2, sbs2, groups2 = _grid(deg_sorted_all, npos)
    dcol = 0
    for sb in sbs2:
        nblk_sb = sb["b1"] - sb["b0"]
        sb["nb4"] = ((nblk_sb + PW - 1) // PW) * PW
        sb["dcol0"] = dcol
        dcol += sb["nb4"] // PW
    dtot = dcol
    # force slot-count per superblock to a multiple of PW so rows pack cleanly
    for sb in sbs2:
        r = (-sb["S"]) % PW
        if r:
            L2[sb["b1"] - 1] += r
            sb["S"] += r
    slot0 = 0
    for sb in sbs2:
        sb["slot0"] = slot0
        slot0 += sb["S"]
    groups2 = []
    for si, sb in enumerate(sbs2):
        off = 0
        b = sb["b0"]
        while b < sb["b1"]:
            b0 = b
            while b < sb["b1"] and L2[b] == L2[b0]:
                b += 1
            groups2.append({"sb": si, "b0": b0, "B": b - b0, "L": L2[b0],
                            "slot_off": off})
            off += (b - b0) * L2[b0]
    S2 = int(sum(L2))
    nblk2 = npos // P

    cores = []
    lo_pack = (K + PW) // PW    # first all-default packed row in the packed view
    for c in range(NCORES):
        sl, dl = core_dat[c]
        eidx, mask, degpos, order, idxflat = _edge_streams(
            sl, dl, rowmap[sl], npos, NPC, L2, sbs2, K + 1
        )
        # pack PW consecutive slots per partition; all-default packs read one
        # PW*512B replica row, mixed packs read an on-device-built pairfix row
        rngp = np.random.default_rng(4242 + c)
        pidx_segs = []
        pfix_vals = []
        for sb in sbs2:
            s0, S = sb["slot0"], sb["S"]
            iv = idxflat[s0 * P:(s0 + S) * P].reshape(S // PW, PW, P)
            pp = rngp.integers(lo_pack, VTAB // PW,
                               (S // PW, P)).astype(np.int16)
            mixed = (iv <= K).any(axis=1)
            nm = int(mixed.sum())
            if nm:
                pp[mixed] = (VTAB // PW + len(pfix_vals) // PW
                             + np.arange(nm)).astype(np.int16)
                mv = np.moveaxis(iv, 1, 2)[mixed].reshape(-1)
                pfix_vals.extend(mv.tolist())
            pidx_segs.append(_wrap16(pp.reshape(-1)))
        pidx = np.concatenate(pidx_segs, axis=1)
        rngd = np.random.default_rng(777 + c)
        dv = rngd.integers(K + 1, VTAB, npos).astype(np.int16)
        dvr = rowmap[c * NPC + order]
        dz = dvr == 0
        dvr = dvr.copy()
        dvr[dz] = rngd.integers(K + 1, VTAB, int(dz.sum())).astype(np.int16)
        dv[:NPC] = dvr
        dsegs = []
        for sb in sbs2:
            nblk_sb = sb["b1"] - sb["b0"]
            nb4 = sb["nb4"]
            vals = np.full((nb4, P), 0, np.int16)
            vals[:nblk_sb] = dv[P * sb["b0"]:P * sb["b1"]].reshape(nblk_sb, P)
            if nb4 > nblk_sb:
                vals[nblk_sb:] = rngd.integers(
                    K + 1, VTAB, (nb4 - nblk_sb, P)).astype(np.int16)
            v4 = vals.reshape(nb4 // PW, PW, P)
            pp = rngd.integers(lo_pack, VTAB // PW,
                               (nb4 // PW, P)).astype(np.int16)
            mixed = (v4 <= K).any(axis=1)
            nm = int(mixed.sum())
            if nm:
                pp[mixed] = (VTAB // PW + len(pfix_vals) // PW
                             + np.arange(nm)).astype(np.int16)
                pfix_vals.extend(np.moveaxis(v4, 1, 2)[mixed].reshape(-1).tolist())
            dsegs.append(_wrap16(pp.reshape(-1)))
        didx = np.concatenate(dsegs, axis=1)
        cores.append({"eidx": eidx, "mask": mask, "degpos": degpos,
                      "didx": didx, "order": order, "pidx": pidx,
                      "pfix": np.asarray(pfix_vals, np.int16)})
    # common pairfix region size across cores (SPMD program is shared)
    npf = max((len(c["pfix"]) for c in cores), default=0)
    Spf = max((npf + P - 1) // P, 1)
    for c in cores:
        pf = np.zeros(Spf * P, np.int16)
        pf[:len(c["pfix"])] = c["pfix"]
        c["pfidx"] = _wrap16(pf)

    meta = {
        "K": K, "K1pos": K1pos, "nblk1": nblk1, "nU": nU, "nUt": nUt,
        "L1": L1, "sbs1": sbs1, "groups1": groups1, "S1": S1,
        "L2": L2, "sbs2": sbs2, "groups2": groups2, "S2": S2, "nblk2": nblk2,
        "npos": npos, "Spf": Spf, "dtot": dtot,
    }
    l1 = {"uidx": uidx, "l1_eidx": l1_eidx, "l1_didx": l1_didx,
          "l1_mask": l1_mask, "l1_degpos": l1_degpos}
    return meta, l1, cores


def _emit_group(nc, gw, Gap, mask_ap, adst_ap, degpos_ap, B, L):
    """Segment softmax + weighted sum for B blocks of equal padded degree L.

    Gap: AP view [128, B*L, 128] of the gathered rows (slot-flat).
    Returns msg tile [128, B, 64]."""
    BL = B * L
    asrc = Gap[:, :, 64:65].rearrange("p s o -> p (s o)")        # [128, BL]
    s_t = gw.tile([P, B, L], F32, tag="s_t")
    nc.vector.tensor_tensor(s_t[:], asrc, adst_ap.to_broadcast((P, B, L)),
                            op=OP.add)
    u_t = gw.tile([P, B, L], F32, tag="u_t")
    nc.vector.scalar_tensor_tensor(u_t[:], s_t[:], NEG_SLOPE, s_t[:],
                                   op0=OP.mult, op1=OP.max)
    e2_t = gw.tile([P, B, L], F32, tag="e2_t")
    nc.vector.scalar_tensor_tensor(e2_t[:], u_t[:], BIG, mask_ap,
                                   op0=OP.add, op1=OP.mult)
    mneg = gw.tile([P, B], F32, tag="mneg")
    nc.vector.tensor_reduce(mneg[:], e2_t[:], axis=mybir.AxisListType.X,
                            op=OP.max, negate=True)
    d_t = gw.tile([P, B, L], F32, tag="d_t")
    nc.vector.tensor_tensor(d_t[:], e2_t[:], mneg[:].to_broadcast((P, B, L)),
                            op=OP.add)
    ex_t = gw.tile([P, B, L], F32, tag="ex_t")
    nc.scalar.activation(ex_t[:], d_t[:], AF.Exp)
    ssum = gw.tile([P, B], F32, tag="ssum")
    nc.vector.tensor_reduce(ssum[:], ex_t[:], axis=mybir.AxisListType.X,
                            op=OP.add)
    sp = gw.tile([P, B], F32, tag="sp")
    nc.vector.tensor_scalar_add(sp[:], ssum[:], EPS)
    rs = gw.tile([P, B], F32, tag="rs")
    nc.vector.reciprocal(rs[:], sp[:])
    rsd = gw.tile([P, B], F32, tag="rsd")
    nc.vector.tensor_tensor(rsd[:], rs[:], degpos_ap, op=OP.mult)
    alpha = gw.tile([P, B, L], F32, tag="alpha")
    nc.vector.tensor_tensor(alpha[:], ex_t[:], rsd[:].to_broadcast((P, B, L)),
                            op=OP.mult)
    wr = gw.tile([P, BL, D], F32, tag="wr")
    nc.vector.tensor_tensor(wr[:], Gap[:, :, 0:D],
                            alpha[:].rearrange("p b l -> p (b l)")
                            .to_broadcast((P, BL, D)), op=OP.mult)
    msg = gw.tile([P, B, D], F32, tag="msg")
    nc.vector.tensor_reduce(msg[:], wr[:].rearrange("p (b l) f -> p b f l", b=B),
                            axis=mybir.AxisListType.X, op=OP.add)
    return msg


def build(meta, repeat=1, limit_sb=None, debug_lvl=3, gchunk=GCHUNK):
    """Build the SPMD Bass program (common across cores)."""
    K = meta["K"]
    nblk1, nUt = meta["nblk1"], meta["nUt"]
    S1, sbs1, groups1, L1 = meta["S1"], meta["sbs1"], meta["groups1"], meta["L1"]
    S2, sbs2, groups2, L2 = meta["S2"], meta["sbs2"], meta["groups2"], meta["L2"]
    nblk2 = meta["nblk2"]

    nc = bacc.Bacc("TRN2", target_bir_lowering=False, debug=False,
                   num_devices=NCORES)
    dt = nc.dram_tensor
    x_in = dt("x_in", [N, D], F32, kind="ExternalInput").ap()
    W1_in = dt("W1_in", [D, D], F32, kind="ExternalInput").ap()
    W1T_in = dt("W1T_in", [D, D], F32, kind="ExternalInput").ap()
    W2_in = dt("W2_in", [D, D], F32, kind="ExternalInput").ap()
    W2T_in = dt("W2T_in", [D, D], F32, kind="ExternalInput").ap()
    av1_in = dt("av1_in", [D, 2], F32, kind="ExternalInput").ap()
    av2_in = dt("av2_in", [D, 2], F32, kind="ExternalInput").ap()
    b1row_in = dt("b1row_in", [1, D], F32, kind="ExternalInput").ap()
    b2row_in = dt("b2row_in", [1, D], F32, kind="ExternalInput").ap()
    b1col_in = dt("b1col_in", [D, 1], F32, kind="ExternalInput").ap()
    uidx_in = dt("uidx_in", [P, nUt], I32, kind="ExternalInput").ap()
    l1_eidx_in = dt("l1_eidx_in", [P, 8 * S1], I16, kind="ExternalInput").ap()
    l1_didx_in = dt("l1_didx_in", [P, 8 * nblk1], I16, kind="ExternalInput").ap()
    l1_mask_in = dt("l1_mask_in", [P, S1], F32, kind="ExternalInput").ap()
    l1_degpos_in = dt("l1_degpos_in", [P, nblk1], F32, kind="ExternalInput").ap()
    Spf = meta["Spf"]
    pidx_in = dt("pidx_in", [P, 8 * (S2 // PW)], I16, kind="ExternalInput").ap()
    pfidx_in = dt("pfidx_in", [P, 8 * Spf], I16, kind="ExternalInput").ap()
    didx_in = dt("didx_in", [P, 8 * meta["dtot"]], I16, kind="ExternalInput").ap()
    mask_in = dt("mask_in", [P, S2], F32, kind="ExternalInput").ap()
    degpos_in = dt("degpos_in", [P, nblk2], F32, kind="ExternalInput").ap()
    out_t = dt("out", [meta["npos"], D], F32, kind="ExternalOutput").ap()

    h1tab = dt("h1tab", [nUt * P, P], F32).ap()
    tab = dt("tab", [VTAB + Spf * P, P], F32).ap()

    with tile.TileContext(nc) as tc, ExitStack() as ctx:
        const = ctx.enter_context(tc.tile_pool(name="const", bufs=1))
        psc_ctx = tc.tile_pool(name="psc", bufs=1, space="PSUM")
        psc = psc_ctx.__enter__()

        ident = const.tile([P, P], F32)
        make_identity(nc, ident[:])

        # ---- weights / augmented matrices ----
        W1s = const.tile([D, D], F32)
        nc.sync.dma_start(W1s[:], W1_in[:])
        W1Ts = const.tile([D, D], F32)
        nc.sync.dma_start(W1Ts[:], W1T_in[:])
        W2s = const.tile([D, D], F32)
        nc.sync.dma_start(W2s[:], W2_in[:])
        W2Ts = const.tile([D, D], F32)
        nc.sync.dma_start(W2Ts[:], W2T_in[:])
        av1s = const.tile([D, 2], F32)
        nc.sync.dma_start(av1s[:], av1_in[:])
        av2s = const.tile([D, 2], F32)
        nc.sync.dma_start(av2s[:], av2_in[:])
        b1cols = const.tile([D, 1], F32)
        nc.sync.dma_start(b1cols[:], b1col_in[:])

        wt1_p = psc.tile([D, 2], F32, space="PSUM")
        nc.tensor.matmul(wt1_p[:], W1Ts[:], av1s[:], start=True, stop=True)
        wt2_p = psc.tile([D, 2], F32, space="PSUM")
        nc.tensor.matmul(wt2_p[:], W2Ts[:], av2s[:], start=True, stop=True)
        wt2s = const.tile([D, 2], F32)
        nc.vector.tensor_copy(wt2s[:], wt2_p[:])

        W1aug = const.tile([D, D + 2], F32)
        nc.vector.tensor_copy(W1aug[:, 0:D], W1s[:])
        nc.vector.tensor_copy(W1aug[:, D:D + 2], wt1_p[:])

        # SPEC [65, 66] = [[I | wt2s | wt2d]; [b1 | b1.wt2s | b1.wt2d]]
        SPEC = const.tile([D + 1, D + 2], F32)
        nc.vector.tensor_copy(SPEC[0:D, 0:D], ident[0:D, 0:D])
        nc.vector.tensor_copy(SPEC[0:D, D:D + 2], wt2s[:])
        nc.sync.dma_start(SPEC[D:D + 1, 0:D], b1row_in[:])
        b1w_p = psc.tile([1, 2], F32, space="PSUM")
        nc.tensor.matmul(b1w_p[:], b1cols[:], wt2s[:], start=True, stop=True)
        nc.vector.tensor_copy(SPEC[D:D + 1, D:D + 2], b1w_p[:])

        W2OUT = const.tile([D + 1, D], F32)
        nc.vector.tensor_copy(W2OUT[0:D, :], W2s[:])
        nc.sync.dma_start(W2OUT[D:D + 1, :], b2row_in[:])

        psc_ctx.__exit__(None, None, None)

        # ---- layer 1: build h1 table for the U endpoint nodes ----
        uidx_s = const.tile([P, nUt], I32)
        nc.sync.dma_start(uidx_s[:], uidx_in[:])
        with tc.tile_pool(name="l1u", bufs=2) as l1u, \
             tc.tile_pool(name="l1up", bufs=2, space="PSUM") as l1up:
            for t in range(nUt):
                xU = l1u.tile([P, D], F32, tag="xU")
                nc.gpsimd.indirect_dma_start(
                    out=xU[:], out_offset=None, in_=x_in[:, :],
                    in_offset=bass.IndirectOffsetOnAxis(ap=uidx_s[:, t:t + 1], axis=0))
                xT_p = l1up.tile([D, P], F32, space="PSUM", tag="xT")
                nc.tensor.transpose(xT_p[:], xU[:], ident[:])
                xT_s = l1u.tile([D, P], F32, tag="xTs")
                nc.vector.tensor_copy(xT_s[:], xT_p[:])
                h_p = l1up.tile([P, D + 2], F32, space="PSUM", tag="h_p")
                nc.tensor.matmul(h_p[:], xT_s[:], W1aug[:], start=True, stop=True)
                h_s = l1u.tile([P, P], F32, tag="h_s")
                nc.scalar.copy(h_s[:, 0:D + 2], h_p[:])
                nc.vector.memset(h_s[:, D + 2:P], 0.0)
                nc.sync.dma_start(h1tab[t * P:(t + 1) * P, :], h_s[:])

        # ---- layer 1 conv -> write table rows ----
        l1_eidx_s = const.tile([P, 8 * S1], I16)
        nc.sync.dma_start(l1_eidx_s[:], l1_eidx_in[:])
        l1_didx_s = const.tile([P, 8 * nblk1], I16)
        nc.sync.dma_start(l1_didx_s[:], l1_didx_in[:])
        l1_mask_s = const.tile([P, S1], F32)
        nc.sync.dma_start(l1_mask_s[:], l1_mask_in[:])
        l1_degpos_s = const.tile([P, nblk1], F32)
        nc.sync.dma_start(l1_degpos_s[:], l1_degpos_in[:])

        with tc.tile_pool(name="l1w", bufs=2) as l1w, \
             tc.tile_pool(name="l1p", bufs=2, space="PSUM") as l1p:
            dr1 = l1w.tile([P, nblk1, P], F32, tag="dr1")
            nc.gpsimd.dma_gather(dr1[:], h1tab[:, :], l1_didx_s[:],
                                 nblk1 * P, nblk1 * P, P, single_packet=False)
            adst1 = l1w.tile([P, nblk1], F32, tag="adst1")
            nc.scalar.activation(adst1[:],
                                 dr1[:, 0:nblk1, 65:66].rearrange("p b o -> p (b o)"),
                                 AF.Identity)
            for sb_i, sb in enumerate(sbs1):
                G1 = l1w.tile([P, sb["S"], P], F32, tag="G1")
                nc.gpsimd.dma_gather(
                    G1[:], h1tab[:, :],
                    l1_eidx_s[:, 8 * sb["slot0"]:8 * (sb["slot0"] + sb["S"])],
                    sb["S"] * P, sb["S"] * P, P, single_packet=False)
                for g in [g for g in groups1 if g["sb"] == sb_i]:
                    B, L, off = g["B"], g["L"], g["slot_off"]
                    sl0 = sb["slot0"] + off
                    msg = _emit_group(
                        nc, l1w, G1[:, off:off + B * L, :],
                        l1_mask_s[:, sl0:sl0 + B * L],
                        adst1[:, g["b0"]:g["b0"] + B],
                        l1_degpos_s[:, g["b0"]:g["b0"] + B], B, L)
                    for j in range(B):
                        b = g["b0"] + j
                        mT_p = l1p.tile([D, P], F32, space="PSUM", tag="mT")
                        nc.tensor.transpose(mT_p[:], msg[:, j, :], ident[:])
                        mT_s = l1w.tile([D + 1, P], F32, tag="mTs")
                        nc.vector.tensor_copy(mT_s[0:D, :], mT_p[:])
                        nc.vector.memset(mT_s[D:D + 1, :], 1.0)
                        row_p = l1p.tile([P, D + 2], F32, space="PSUM", tag="rowp")
                        nc.tensor.matmul(row_p[:], mT_s[:], SPEC[:],
                                         start=True, stop=True)
                        row_s = l1w.tile([P, P], F32, tag="rows")
                        nc.scalar.copy(row_s[:, 0:D + 2], row_p[:])
                        nc.vector.memset(row_s[:, D + 2:P], 0.0)
                        nrows = min(P, K - b * P)
                        if nrows > 0:
                            nc.sync.dma_start(
                                tab[1 + b * P:1 + b * P + nrows, :],
                                row_s[0:nrows, :])
                        if b == K // P:   # default row from the pad position K
                            q = K % P
                            nc.sync.dma_start(tab[0:1, :], row_s[q:q + 1, :])
                            # replicate the default row over rows K+1..VTAB-1
                            # (spreads the 99%-default gather traffic across
                            # HBM addresses instead of hammering one row)
                            zidx = l1w.tile([P, 8], I16, tag="zidx")
                            nc.vector.memset(zidx[:], 0)
                            defbc = l1w.tile([P, 1, P], F32, tag="defbc")
                            nc.gpsimd.dma_gather(defbc[:], tab[:, :], zidx[:],
                                                 P, P, P, single_packet=False)
                            r0 = K + 1
                            while r0 < VTAB:
                                cnt = min(P, VTAB - r0)
                                nc.sync.dma_start(tab[r0:r0 + cnt, :],
                                                  defbc[0:cnt, 0, :])
                                r0 += cnt

        # ---- build pairfix rows: [row(a) | row(b)] for mixed pairs ----
        pfidx_s = const.tile([P, 8 * Spf], I16)
        nc.sync.dma_start(pfidx_s[:], pfidx_in[:])
        with tc.tile_pool(name="pfw", bufs=1) as pfw:
            pfg = pfw.tile([P, Spf, P], F32)
            nc.gpsimd.dma_gather(pfg[:], tab[0:VTAB, :], pfidx_s[:],
                                 Spf * P, Spf * P, P, single_packet=False)
            nc.sync.dma_start(
                tab[VTAB:VTAB + Spf * P, :].rearrange("(s p) f -> p s f", p=P),
                pfg[:])

        # ---- layer 2 ----
        tp = tab[:].rearrange("(r w) f -> r (w f)", w=PW)
        pidx_s = const.tile([P, 8 * (S2 // PW)], I16)
        nc.sync.dma_start(pidx_s[:], pidx_in[:])
        didx_s = const.tile([P, 8 * meta["dtot"]], I16)
        nc.sync.dma_start(didx_s[:], didx_in[:])
        mask_s = const.tile([P, S2], F32)
        nc.sync.dma_start(mask_s[:], mask_in[:])
        degpos_s = const.tile([P, nblk2], F32)
        nc.sync.dma_start(degpos_s[:], degpos_in[:])

        with tc.tile_pool(name="sbw", bufs=2) as sbw, \
             tc.tile_pool(name="gw", bufs=2) as gw, \
             tc.tile_pool(name="blk", bufs=3) as blk, \
             tc.tile_pool(name="psb", bufs=3, space="PSUM") as psb:
            sbs2_run = sbs2 if limit_sb is None else sbs2[:limit_sb]
            for _rep in range(repeat):
                for sb_i, sb in enumerate(sbs2_run):
                    nblk_sb = sb["b1"] - sb["b0"]
                    hS = sb["S"] // PW
                    pidx_t = pidx_s[:, 8 * (sb["slot0"] // PW):
                                    8 * (sb["slot0"] // PW + hS)]
                    G = sbw.tile([P, sb["S"], P], F32, tag="G")
                    for off in range(0, sb["S"], gchunk):
                        cs = min(gchunk, sb["S"] - off)
                        Gv = G[:, off:off + cs, :].rearrange(
                            "p (k w) f -> p k (w f)", w=PW)
                        nc.gpsimd.dma_gather(
                            Gv, tp, pidx_t[:, 8 * (off // PW):
                                           8 * ((off + cs) // PW)],
                            cs // PW * P, cs // PW * P, PW * P,
                            single_packet=False)
                    nb4 = sb["nb4"]
                    dr = sbw.tile([P, nb4, P], F32, tag="dr")
                    nc.gpsimd.dma_gather(
                        dr[:].rearrange("p (k w) f -> p k (w f)", w=PW), tp,
                        didx_s[:, 8 * sb["dcol0"]:8 * (sb["dcol0"] + nb4 // PW)],
                        nb4 // PW * P, nb4 // PW * P, PW * P,
                        single_packet=False)
                    adst = sbw.tile([P, nblk_sb], F32, tag="adst")
                    nc.scalar.activation(
                        adst[:],
                        dr[:, 0:nblk_sb, 65:66].rearrange("p b o -> p (b o)"),
                        AF.Identity)
                    if debug_lvl < 2:
                        dum = sbw.tile([P, P], F32, tag="dum")
                        nc.vector.tensor_copy(dum[:], G[:, 0, :])
                        continue
                    for g in [g for g in groups2 if g["sb"] == sb_i]:
                        B, L, off = g["B"], g["L"], g["slot_off"]
                        sl0 = sb["slot0"] + off
                        msg = _emit_group(
                            nc, gw, G[:, off:off + B * L, :],
                            mask_s[:, sl0:sl0 + B * L],
                            adst[:, g["b0"] - sb["b0"]:g["b0"] - sb["b0"] + B],
                            degpos_s[:, g["b0"]:g["b0"] + B], B, L)
                        if debug_lvl < 3:
                            dum2 = blk.tile([P, D], F32, tag="dum2")
                            nc.vector.tensor_copy(dum2[:], msg[:, 0, :])
                            continue
                        for j in range(B):
                            b = g["b0"] + j
                            mT_p = psb.tile([D, P], F32, space="PSUM", tag="mT")
                            nc.tensor.transpose(mT_p[:], msg[:, j, :], ident[:])
                            mT_s = blk.tile([D + 1, P], F32, tag="mTs")
                            nc.vector.tensor_copy(mT_s[0:D, :], mT_p[:])
                            nc.vector.memset(mT_s[D:D + 1, :], 1.0)
                            o_p = psb.tile([P, D], F32, space="PSUM", tag="op")
                            nc.tensor.matmul(o_p[:], mT_s[:], W2OUT[:],
                                             start=True, stop=True)
                            o_s = blk.tile([P, D], F32, tag="os")
                            nc.scalar.copy(o_s[:], o_p[:])
                            nc.sync.dma_start(out_t[b * P:(b + 1) * P, :], o_s[:])

    nc.compile()
    return nc


def make_in_maps(inputs, meta, l1, cores):
    x = np.ascontiguousarray(np.asarray(inputs["x"], dtype=np.float32))
    W1 = np.asarray(inputs["W1"], dtype=np.float32)
    W2 = np.asarray(inputs["W2"], dtype=np.float32)
    base = {
        "x_in": x,
        "W1_in": np.ascontiguousarray(W1),
        "W1T_in": np.ascontiguousarray(W1.T),
        "W2_in": np.ascontiguousarray(W2),
        "W2T_in": np.ascontiguousarray(W2.T),
        "av1_in": np.ascontiguousarray(np.stack(
            [np.asarray(inputs["a_src1"]), np.asarray(inputs["a_dst1"])],
            axis=1).astype(np.float32)),
        "av2_in": np.ascontiguousarray(np.stack(
            [np.asarray(inputs["a_src2"]), np.asarray(inputs["a_dst2"])],
            axis=1).astype(np.float32)),
        "b1row_in": np.asarray(inputs["b1"], dtype=np.float32).reshape(1, D),
        "b2row_in": np.asarray(inputs["b2"], dtype=np.float32).reshape(1, D),
        "b1col_in": np.asarray(inputs["b1"], dtype=np.float32).reshape(D, 1),
        "uidx_in": l1["uidx"],
        "l1_eidx_in": l1["l1_eidx"],
        "l1_didx_in": l1["l1_didx"],
        "l1_mask_in": l1["l1_mask"],
        "l1_degpos_in": l1["l1_degpos"],
    }
    in_maps = []
    for c in range(NCORES):
        m = dict(base)
        m["pidx_in"] = cores[c]["pidx"]
        m["pfidx_in"] = cores[c]["pfidx"]
        m["didx_in"] = cores[c]["didx"]
        m["mask_in"] = cores[c]["mask"]
        m["degpos_in"] = cores[c]["degpos"]
        in_maps.append(m)
    return in_maps


def unshard(results, cores):
    out = np.empty((N, D), np.float32)
    for c in range(NCORES):
        oc = results[c]["out"]
        order = cores[c]["order"]
        out[c * NPC + order] = oc[:NPC]
    return out


def kernel(**inputs):
    meta, l1, cores = prep(inputs)
    nc = build(meta, repeat=1)
    in_maps = make_in_maps(inputs, meta, l1, cores)
    res = run_bass_kernel_spmd(nc, in_maps, core_ids=list(range(NCORES)))
    return unshard(res.results, cores)

